# revision 1
# baseline (speedup 1.0000x reference)
"""SimCLR (NT-Xent) contrastive loss on 8 TRN2 NeuronCores.

reference semantics:
    xn = x / max(||x||, eps);  sim = xn @ xn.T;  sim[i,i] = -inf
    logits = sim / 0.5;  target(i) = i ^ 1
    loss = mean_i( logsumexp(logits[i,:]) - logits[i, target(i)] )

Distribution: data-parallel over rows of the similarity matrix. Each core
receives the full x^T (moving operand, bf16, pre-tiled [nt][p][k][n] so
every DMA is contiguous per partition) plus its own 512-column slice
(stationary operand), so the SPMD graph is identical on every core.

No collectives: the 8-rank ncfw AllGather has a ~10-20us/step latency
floor here (~75us total) that parked the TensorEngine. Instead every core
computes all 4096 squared norms itself from the [128,128] diagonal blocks
of the raw Gram matrix (+13.6us of PE), broadcasts 1/norm to all
partitions via a DRAM round-trip + stride-0 DMA, and every S block is
drained PSUM->SBUF (bf16) immediately so the PE never waits on PSUM or on
the norm pipeline. Epilogue = DVE column-scale + one ACT exp-with-rowsum
per block. Host sums the 8 per-core partial losses.
"""

import numpy as np

try:
    import concourse.bass as bass
except ImportError:  # pragma: no cover
    import sys

    sys.path.insert(0, "/opt/trn_rl_repo")
    import concourse.bass as bass

import ml_dtypes
import concourse.mybir as mybir
from concourse import bacc, tile
from concourse.bass_utils import run_bass_kernel_spmd

B, D, NCORES = 4096, 1024, 8
RPC = B // NCORES  # rows per core (512)
KT = D // 128  # contraction chunks (8)
NT = B // 512  # moving-operand column tiles (8)
RC = RPC // 128  # 128-row chunks per core (4)
E2 = 7.38905609893065  # exp(sim_ii / T) with sim_ii == 1
F32 = mybir.dt.float32
BF16 = mybir.dt.bfloat16


def build(stage="full"):
    Act = mybir.ActivationFunctionType
    nc = bacc.Bacc("TRN2", target_bir_lowering=False, num_devices=NCORES)

    xt = nc.dram_tensor("xt", [NT, 128, KT, 512], BF16, kind="ExternalInput")
    xo = nc.dram_tensor("xo", [128, KT, RPC], BF16, kind="ExternalInput")
    diagmask = nc.dram_tensor("diagmask", [128, 512], F32, kind="ExternalInput")
    pairmask = nc.dram_tensor("pairmask", [128, 128], F32, kind="ExternalInput")
    out = nc.dram_tensor("out", [1, 1], F32, kind="ExternalOutput")

    rn_d = nc.dram_tensor("rn_d", [B], F32, kind="Internal")

    with tile.TileContext(nc) as tc:
        with (
            tc.tile_pool(name="sb", bufs=1) as sb,
            tc.tile_pool(name="ps", bufs=7, space="PSUM") as psp,
            tc.tile_pool(name="aux", bufs=1, space="PSUM") as auxp,
        ):
            # ---- persistent SBUF tensors ----
            xo_sb = sb.tile([128, KT, RPC], BF16, tag="xo")
            strips = [
                sb.tile([128, KT, 512], BF16, tag=f"strip{i}", name=f"strip{i}")
                for i in range(NT)
            ]
            sdef = [
                sb.tile([128, 512], BF16, tag=f"sdef{i}", name=f"sdef{i}")
                for i in range(RC * NT)
            ]
            dmask = sb.tile([128, 512], F32, tag="dmask")
            pmask = sb.tile([128, 128], F32, tag="pmask")
            rn_bc = sb.tile([128, B], F32, tag="rnbc")
            ones128 = sb.tile([128, 1], F32, tag="ones128")
            n2 = sb.tile([128, RC], F32, tag="n2")
            n2r = sb.tile([128, RC], F32, tag="n2r")
            rn_loc = sb.tile([128, RC], F32, tag="rnloc")
            rn2_loc = sb.tile([128, RC], F32, tag="rn2loc")
            rn_swap = sb.tile([128, RC], F32, tag="rnswap")
            pairv = sb.tile([128, RC], F32, tag="pairv")
            n2a = sb.tile([128, RC * NT], F32, tag="n2a")
            n2ar = sb.tile([128, RC * NT], F32, tag="n2ar")
            rn_all = sb.tile([128, RC * NT], F32, tag="rnall")
            zacc = sb.tile([128, RC * NT], F32, tag="zacc")

            # ---- input DMA: two HWDGE issue streams (SP + ACT sequencers);
            # tiny masks first so extracts are never gated on bulk data
            nc.sync.dma_start(dmask[:], diagmask[:])
            nc.sync.dma_start(pmask[:], pairmask[:])
            for ntb in range(0, NT, 2):
                nc.sync.dma_start(strips[ntb][:], xt[ntb])
            nc.scalar.dma_start(xo_sb[:], xo[:])
            for ntb in range(1, NT, 2):
                nc.scalar.dma_start(strips[ntb][:], xt[ntb])
            nc.vector.memset(ones128[:], 1.0)
            neg_e2 = sb.tile([128, 1], F32, tag="nege2")
            nc.vector.memset(neg_e2[:], -E2)

            # ---- phase A: own diagonal blocks -> own rn + pair sims ----
            for rc in range(RC):
                psA = psp.tile([128, 128], F32, tag="ps")
                own = xo_sb[:, :, rc * 128 : (rc + 1) * 128]
                for k in range(KT):
                    nc.tensor.matmul(
                        psA[:],
                        own[:, k, :],
                        own[:, k, :],
                        start=(k == 0),
                        stop=(k == KT - 1),
                    )
                jd = sb.tile([128, 128], F32, tag="junk128", bufs=2)
                nc.vector.tensor_mul(jd[:], psA[:], dmask[:, 0:128])
                nc.vector.reduce_sum(
                    n2[:, rc : rc + 1], jd[:], axis=mybir.AxisListType.X
                )
                jp = sb.tile([128, 128], F32, tag="junk128", bufs=2)
                nc.vector.tensor_mul(jp[:], psA[:], pmask[:])
                nc.vector.reduce_sum(
                    pairv[:, rc : rc + 1], jp[:], axis=mybir.AxisListType.X
                )

            # rn = 1/sqrt(n2) (DVE reciprocal + ACT sqrt; ACT rsqrt is banned)
            nc.vector.reciprocal(n2r[:], n2[:])
            nc.scalar.activation(rn_loc[:], n2r[:], Act.Sqrt)
            nc.vector.tensor_scalar_mul(rn2_loc[:], rn_loc[:], 2.0)

            # partner-swapped rn via pair-permutation matmul
            psS = auxp.tile([128, RC], F32, tag="aux")
            nc.tensor.matmul(psS[:], pmask[:], rn_loc[:], start=True, stop=True)
            nc.vector.tensor_copy(rn_swap[:], psS[:])

            # ---- phase C. Two kinds of PE work per strip: a cheap global
            # diagonal block (feeds the global norms) and 4 S row-blocks.
            # Diagonal blocks are emitted as early as possible (they are
            # DMA-paced), interleaved with main blocks so the rn pipeline
            # completes while most S matmuls are still running. Every PSUM
            # block is drained to bf16 SBUF immediately, so the PE never
            # waits on PSUM or on the norm pipeline.
            def d_block(ntb):
                psD = psp.tile([128, 512], F32, tag="ps", name="psD")
                for sub in range(RC):
                    seg = strips[ntb][:, :, sub * 128 : (sub + 1) * 128]
                    for k in range(KT):
                        nc.tensor.matmul(
                            psD[:, sub * 128 : (sub + 1) * 128],
                            seg[:, k, :],
                            seg[:, k, :],
                            start=(k == 0),
                            stop=(k == KT - 1),
                        )
                jq = sb.tile([128, 512], F32, tag="junk512", bufs=2, name="jq")
                nc.vector.tensor_mul(jq[:], psD[:], dmask[:])
                nc.vector.reduce_sum(
                    n2a[:, ntb * RC : (ntb + 1) * RC],
                    jq[:].rearrange("p (a b) -> p a b", b=128),
                    axis=mybir.AxisListType.X,
                )

            def c_strip(ntb, fused_epilogue=None):
                for rcb in range(RC):
                    ps = psp.tile([128, 512], F32, tag="ps", name="psC")
                    for k in range(KT):
                        nc.tensor.matmul(
                            ps[:],
                            xo_sb[:, k, rcb * 128 : (rcb + 1) * 128],
                            strips[ntb][:, k, :],
                            start=(k == 0),
                            stop=(k == KT - 1),
                        )
                    if fused_epilogue is None:
                        nc.vector.tensor_copy(sdef[rcb * NT + ntb][:], ps[:])
                    else:
                        fused_epilogue(ps[:], rcb, ntb)

            # all diagonal blocks first: they are cheap and DMA-paced, and
            # the global-norm pipeline completes while the 32 main S blocks
            # (emitted after) still have ~50us of PE work left
            for ntb in range(NT):
                d_block(ntb)

            # global rn: 1/sqrt, then DRAM round-trip in PARTITION-MAJOR
            # order (contiguous 128B/partition store instead of a 15us
            # 4-byte-strided scatter) + chunked stride-0 partition bcast.
            # rn_bc column c = 32*p + b (b = global 128-row block); the
            # epilogue undoes the permutation with a strided AP for free.
            nc.vector.reciprocal(n2ar[:], n2a[:])
            nc.scalar.activation(rn_all[:], n2ar[:], Act.Sqrt)
            nc.gpsimd.dma_start(
                rn_d.rearrange("(p b) -> p b", p=128), rn_all[:]
            )
            rn_dv = rn_d.rearrange("(a n) -> a n", a=1)
            for q in range(4):
                nc.gpsimd.dma_start(
                    rn_bc[:, q * 1024 : (q + 1) * 1024],
                    rn_dv[:, q * 1024 : (q + 1) * 1024].to_broadcast(
                        [128, 1024]
                    ),
                )
            # rn_bc viewed [q, b(32), p(128)] in strip-column order
            rn_bc_perm = rn_bc[:].rearrange("q (p b) -> q b p", b=32)

            def ep_block(src_ap, rcb, ntb):
                col = rcb * NT + ntb
                scr = sb.tile([128, 512], F32, tag="scr", bufs=3, name="scr")
                nc.vector.tensor_mul(
                    scr[:].rearrange("p (a b) -> p a b", b=128),
                    src_ap.rearrange("p (a b) -> p a b", b=128),
                    rn_bc_perm[:, ntb * RC : (ntb + 1) * RC, :],
                )
                jk = sb.tile([128, 512], F32, tag="junk512", bufs=2, name="jk")
                nc.scalar.activation(
                    jk[:],
                    scr[:],
                    Act.Exp,
                    scale=rn2_loc[:, rcb : rcb + 1],
                    accum_out=zacc[:, col : col + 1],
                )

            def epilogue(ntb):
                for rcb in range(RC):
                    ep_block(sdef[rcb * NT + ntb][:], rcb, ntb)

            # strips 0-1: matmuls land before the norm pipeline finishes ->
            # drain to SBUF and defer their epilogues. Strips 2-7 complete
            # after rn_bc is ready, so their epilogues run fused straight
            # from PSUM and the exp stream starts ~15us earlier.
            c_strip(0)
            c_strip(1)
            c_strip(2, fused_epilogue=ep_block)
            epilogue(0)
            c_strip(3, fused_epilogue=ep_block)
            epilogue(1)
            for ntb in range(4, NT):
                c_strip(ntb, fused_epilogue=ep_block)

            # ---- phase D: per-row loss and final reduction ----
            zview = zacc[:].rearrange("p (a b) -> p a b", b=NT)
            zrow = sb.tile([128, RC], F32, tag="zrow")
            nc.vector.reduce_sum(zrow[:], zview, axis=mybir.AxisListType.X)
            lv = sb.tile([128, RC], F32, tag="lv")
            nc.scalar.activation(lv[:], zrow[:], Act.Ln, bias=neg_e2[:])
            t1 = sb.tile([128, RC], F32, tag="t1")
            nc.vector.tensor_mul(t1[:], pairv[:], rn_loc[:])
            t2 = sb.tile([128, RC], F32, tag="t2")
            nc.vector.tensor_mul(t2[:], t1[:], rn_swap[:])
            t3 = sb.tile([128, RC], F32, tag="t3")
            nc.vector.tensor_scalar_mul(t3[:], t2[:], 2.0)
            lossv = sb.tile([128, RC], F32, tag="lossv")
            nc.vector.tensor_sub(lossv[:], lv[:], t3[:])
            ltot = sb.tile([128, 1], F32, tag="ltot")
            nc.vector.reduce_sum(ltot[:], lossv[:], axis=mybir.AxisListType.X)
            psF = auxp.tile([1, 1], F32, tag="aux", name="psF")
            nc.tensor.matmul(psF[:], ones128[:], ltot[:], start=True, stop=True)
            osb = sb.tile([1, 1], F32, tag="osb", name="osb")
            nc.vector.tensor_copy(osb[:], psF[:])
            nc.sync.dma_start(out[:], osb[:])

    nc.finalize()  # run bacc passes (register allocation etc.)
    return nc


_CACHE = {}


def get_built(stage="full"):
    if stage not in _CACHE:
        _CACHE[stage] = build(stage)
    return _CACHE[stage]


def make_in_maps(image: np.ndarray):
    image = np.asarray(image, dtype=np.float32)
    imT = np.ascontiguousarray(image.T).astype(ml_dtypes.bfloat16)  # [D, B]
    # [D, B] -> [KT, 128, NT, 512] -> tiled [NT, 128, KT, 512]
    xt_t = np.ascontiguousarray(
        imT.reshape(KT, 128, NT, 512).transpose(2, 1, 0, 3)
    )
    idx = np.arange(128)
    dmask = np.tile(np.eye(128, dtype=np.float32), (1, RC))  # [128, 512]
    pmask = np.zeros((128, 128), dtype=np.float32)
    pmask[idx, idx ^ 1] = 1.0
    in_maps = []
    for c in range(NCORES):
        xo_t = np.ascontiguousarray(xt_t[c])
        in_maps.append(
            {"xt": xt_t, "xo": xo_t, "diagmask": dmask, "pairmask": pmask}
        )
    return in_maps


def run(image: np.ndarray, stage="full", **spmd_kwargs):
    nc = get_built(stage)
    in_maps = make_in_maps(image)
    res = run_bass_kernel_spmd(
        nc, in_maps, core_ids=list(range(NCORES)), **spmd_kwargs
    )
    total = sum(float(r["out"][0, 0]) for r in res.results)
    return np.array(total / B, dtype=np.float32), res


def kernel(image: np.ndarray) -> np.ndarray:
    loss, _ = run(image)
    return loss



# revision 14
# speedup vs baseline: 2.0655x; 2.0655x over previous
"""SimCLR (NT-Xent) contrastive loss on 8 TRN2 NeuronCores — fp8 edition.

reference semantics:
    xn = x / max(||x||, eps);  sim = xn @ xn.T;  sim[i,i] = -inf
    logits = sim / 0.5;  target(i) = i ^ 1
    loss = mean_i( logsumexp(logits[i,:]) - logits[i, target(i)] )

Distribution: data-parallel over rows of the similarity matrix. The host
normalizes rows (the sharding hint's "shard normalized x"), scales by 32
and quantizes to fp8e4 (e4m3), so the device computes S' = (32 xn)(32 xn)^T
with DoubleRow fp8 matmuls (2 k-rows/cycle, 2x bf16 throughput) and
logits = S'/512. Each core gets the full x^T pre-tiled [nt][p][k][n] with
its OWN 512-column strip rolled to slot 0, so slot 0 is both the stationary
operand and the diagonal block — one SPMD graph for all cores, no xo copy.

Per strip: 16 DoubleRow matmuls accumulate a [128, 4, 512] PSUM group
(4 banks, double buffered = all 8 banks), one 4-bank ACT exp -> fp16 SBUF,
one DVE fast-mode segmented reduce -> per-block row sums. Pair logits are
pulled from the slot-0 diagonal blocks with fused tensor_tensor_reduce.
Diagonal self-terms are removed via the constant-E2 bias on the final Ln
(S'_ii/512 = 2 +- 0.2%, error ~1e-5 of the row sum). Host sums the 8
per-core [128, 4] loss vectors.
"""

import numpy as np

try:
    import concourse.bass as bass
except ImportError:  # pragma: no cover
    import sys

    sys.path.insert(0, "/opt/trn_rl_repo")
    import concourse.bass as bass

import ml_dtypes
import concourse.mybir as mybir
from concourse import bacc, tile
from concourse.bass_utils import run_bass_kernel_spmd

B, D, NCORES = 4096, 1024, 8
RPC = B // NCORES  # rows per core (512)
KT = D // 128  # contraction chunks (8)
NT = B // 512  # moving-operand column tiles (8)
RC = RPC // 128  # 128-row chunks per core (4)
QSCALE = 32.0  # fp8 pre-scale: quantize 32*xn (keeps entries in e4m3 normals)
SCALE = 2.0 / (QSCALE * QSCALE)  # logits = SCALE * S'
E2 = 7.38905609893065  # exp(logit_ii) with sim_ii == 1
F32 = mybir.dt.float32
FP16 = mybir.dt.float16
FP8 = mybir.dt.float8e4


def build(stage="full"):
    Act = mybir.ActivationFunctionType
    Alu = mybir.AluOpType
    DR = mybir.MatmulPerfMode.DoubleRow
    nc = bacc.Bacc("TRN2", target_bir_lowering=False, num_devices=NCORES)

    xt = nc.dram_tensor("xt", [NT, 128, KT, 512], FP8, kind="ExternalInput")
    pairmask = nc.dram_tensor("pairmask", [128, 128], F32, kind="ExternalInput")
    out = nc.dram_tensor("out", [128, RC], F32, kind="ExternalOutput")

    with tile.TileContext(nc) as tc:
        with (
            tc.tile_pool(name="sb", bufs=1) as sb,
            tc.tile_pool(name="ps", bufs=2, space="PSUM") as psp,
        ):
            strips = [
                sb.tile([128, KT, 512], FP8, tag=f"strip{i}", name=f"strip{i}")
                for i in range(NT)
            ]
            pmask = sb.tile([128, 128], F32, tag="pmask")
            pairv = sb.tile([128, RC], F32, tag="pairv")
            zacc = sb.tile([128, NT * RC], FP16, tag="zacc")
            sc_exp = sb.tile([128, 1], F32, tag="scexp")
            neg_e2 = sb.tile([128, 1], F32, tag="nege2")
            zbias = sb.tile([128, 1], F32, tag="zbias")

            # ---- input DMA on two HWDGE issue streams (SP + ACT
            # sequencers), half-strip granularity round-robin so strips
            # land in issue order faster than the PE consumes them (keeps
            # the PE p-state ramped). Tiny mask first.
            nc.sync.dma_start(pmask[:], pairmask[:])
            nc.vector.memset(sc_exp[:], SCALE)
            nc.vector.memset(neg_e2[:], -E2)
            nc.vector.memset(zbias[:], 0.0)
            qs = [nc.sync, nc.scalar]
            qi = 0
            for i in range(NT):
                for h in range(2):
                    k0, k1 = h * (KT // 2), (h + 1) * (KT // 2)
                    qs[qi % 2].dma_start(
                        strips[i][:, k0:k1, :], xt[i][:, k0:k1, :]
                    )
                    qi += 1

            own = strips[0]
            for ntb in range(NT):
                ps = psp.tile([128, RC, 512], F32, tag="ps", name="ps")
                for rcb in range(RC):
                    stat = own[:, :, rcb * 128 : (rcb + 1) * 128]
                    for kp in range(KT // 2):
                        nc.tensor.matmul(
                            ps[:, rcb, :],
                            stat[:, 2 * kp : 2 * kp + 2, :],
                            strips[ntb][:, 2 * kp : 2 * kp + 2, :],
                            start=(kp == 0),
                            stop=(kp == KT // 2 - 1),
                            perf_mode=DR,
                        )
                et = sb.tile([128, RC, 512], FP16, tag="et", bufs=3, name="et")
                nc.scalar.activation(et[:], ps[:], Act.Exp, scale=sc_exp[:])
                if ntb == 0:
                    # exp(pair logit) sits in the diagonal 128-blocks of
                    # slot 0; gather from the fp16 exp tile (SBUF — a DVE
                    # read of PSUM wedges the device) and undo with Ln
                    for rcb in range(RC):
                        jp = sb.tile([128, 128], F32, tag="jp", bufs=2, name="jp")
                        nc.vector.tensor_mul(
                            jp[:],
                            et[:, rcb, rcb * 128 : (rcb + 1) * 128],
                            pmask[:],
                        )
                        nc.vector.reduce_sum(
                            pairv[:, rcb : rcb + 1],
                            jp[:],
                            axis=mybir.AxisListType.X,
                        )
                with nc.allow_low_precision(
                    reason="fp16 per-block rowsums: 512-term partials, "
                    "~1e-3 relative rounding -> <1e-4 on the final loss"
                ):
                    nc.vector.reduce_sum(
                        zacc[:, ntb * RC : (ntb + 1) * RC],
                        et[:],
                        axis=mybir.AxisListType.X,
                    )

            # ---- per-row loss; host sums the [128, RC] outputs
            zaccf = sb.tile([128, NT * RC], F32, tag="zaccf")
            nc.vector.tensor_copy(zaccf[:], zacc[:])
            zrow = sb.tile([128, RC], F32, tag="zrow")
            nc.vector.reduce_sum(
                zrow[:],
                zaccf[:].rearrange("p (n r) -> p r n", r=RC),
                axis=mybir.AxisListType.X,
            )
            lv = sb.tile([128, RC], F32, tag="lv")
            nc.scalar.activation(lv[:], zrow[:], Act.Ln, bias=neg_e2[:])
            lnp = sb.tile([128, RC], F32, tag="lnp")
            nc.scalar.activation(lnp[:], pairv[:], Act.Ln, bias=zbias[:])
            lossv = sb.tile([128, RC], F32, tag="lossv")
            nc.vector.tensor_sub(lossv[:], lv[:], lnp[:])
            nc.sync.dma_start(out[:], lossv[:])

    nc.finalize()
    return nc


_CACHE = {}


def get_built(stage="full"):
    if stage not in _CACHE:
        _CACHE[stage] = build(stage)
    return _CACHE[stage]


def make_in_maps(image: np.ndarray):
    image = np.asarray(image, dtype=np.float32)
    norms = np.maximum(
        np.sqrt((image * image).sum(axis=1, keepdims=True)), 1e-8
    )
    xq = ((image / norms) * QSCALE).astype(ml_dtypes.float8_e4m3)
    # [B, D] -> [D, B] -> [KT, 128, NT, 512] -> tiled [NT, 128, KT, 512]
    xt_t = np.ascontiguousarray(
        xq.T.reshape(KT, 128, NT, 512).transpose(2, 1, 0, 3)
    )
    idx = np.arange(128)
    pmask = np.zeros((128, 128), dtype=np.float32)
    pmask[idx, idx ^ 1] = 1.0
    in_maps = []
    for c in range(NCORES):
        # own strip rolled to slot 0: slot s holds global strip (c + s) % NT
        xt_c = np.ascontiguousarray(np.roll(xt_t, -c, axis=0))
        in_maps.append({"xt": xt_c, "pairmask": pmask})
    return in_maps


def run(image: np.ndarray, stage="full", **spmd_kwargs):
    nc = get_built(stage)
    in_maps = make_in_maps(image)
    res = run_bass_kernel_spmd(
        nc, in_maps, core_ids=list(range(NCORES)), **spmd_kwargs
    )
    total = sum(float(r["out"].astype(np.float64).sum()) for r in res.results)
    return np.array(total / B, dtype=np.float32), res


def kernel(image: np.ndarray) -> np.ndarray:
    loss, _ = run(image)
    return loss
